# revision 1
# baseline (speedup 1.0000x reference)
"""Distributed causal multi-head attention for Trainium2 (8 NeuronCores).

Problem: x[2,2048,1024] @ w_qkv[1024,3072] -> 16-head causal attention
         -> @ w_out[1024,1024]. fp32 reference; device compute in bf16
         (fp32 PSUM accumulation).

Sharding (8 cores): core c owns heads {2c, 2c+1} for BOTH batches.
Phase 2 is split into two per-head passes (pass A = head 2c, pass B =
head 2c+1) so the all-to-all can be split in two: A2A#1 (head-A
features) runs overlapped with pass B's compute; only A2A#2 is exposed
at the tail. The output projection accumulates in two rounds (head-A
features, then head-B features) so round A also overlaps A2A#2.

Device pipeline per core:
  P1: qT,kT = (w_qk stationary) @ xT chunks   [bf16, N=512 moving]
      vT    = (w_v stationary)  @ xT chunks -> PE-transpose -> V seq-major
      vaug  = [ones | pad | V_h] per j-tile   [ones row 0 => denominators]
      batch 0 runs dt-outer accumulation passes so PE overlaps the xT DMA.
  P2 (per pass h, batch b, i-chunk of 512): software-pipelined causal
      j-tiles: S^T[j,i] (K=64) -> ACT exp (scale fused, bf16) -> diagonal
      mask mul -> PV: pv[128,512] += vaug.T @ P^T (row 0 = denom).
      normalize: recip(denom row 0) -> gpsimd partition-broadcast ->
      multiply -> DMA to per-pass A2A buffer.
  P3: two AllToAll [8,64,512] bf16; out[512,1024] accumulated over two
      rounds of 4 K-tiles each (PSUM round A copied to SBUF, round B
      added on DVE), written back as bf16.
"""
import os
import numpy as np
import ml_dtypes

import concourse.bass as bass
import concourse.bacc as bacc
import concourse.mybir as mybir
import concourse.tile as tile
from concourse.bass_utils import run_bass_kernel_spmd

F32 = mybir.dt.float32
BF16 = mybir.dt.bfloat16
AF = mybir.ActivationFunctionType

NC = 8           # cores
NB = 2           # batches
N = 2048         # seq len
D = 1024         # model dim
HPC = 2          # heads per core
HD = 64          # head dim
FS = HPC * HD    # per-core feature slice (128)
NFLAT = NB * N   # 4096 flattened rows
ROWS = NFLAT // NC   # 512 output rows per core
SCALE = HD ** -0.5

_CACHED_NC = None


def build_graph():
    nc = bacc.Bacc("TRN2", target_bir_lowering=False, debug=False,
                   num_devices=NC)

    xT = nc.dram_tensor("xT", [128, NB, 8, N], BF16, kind="ExternalInput")
    wqkv = nc.dram_tensor("wqkv", [128, 8, 3 * FS], BF16, kind="ExternalInput")
    wout = nc.dram_tensor("wout", [2, 128, 4, D], BF16, kind="ExternalInput")
    mask = nc.dram_tensor("mask", [128, 2, 2, 512], BF16, kind="ExternalInput")
    ident = nc.dram_tensor("ident", [128, 128], BF16, kind="ExternalInput")
    out = nc.dram_tensor("out", [ROWS, D], BF16, kind="ExternalOutput")

    with tile.TileContext(nc) as tc:
        _emit(nc, tc, xT, wqkv, wout, mask, ident, out)
    nc.compile()
    return nc


def _emit(nc, tc, xT, wqkv, wout, mask, ident, out):
    ctx_pools = []

    def pool(name, **kw):
        cm = tc.tile_pool(name=name, **kw)
        p = cm.__enter__()
        ctx_pools.append(cm)
        return p

    wpool = pool("weights", bufs=1)
    ptpool = pool("pt", bufs=8)
    spool = pool("stage", bufs=1)
    dpool = pool("dram", bufs=1, space="DRAM")
    pinit_cm = tc.tile_pool(name="psum_init", bufs=1, space="PSUM")
    pinit = pinit_cm.__enter__()

    # ---- persistent SBUF buffers ----
    xt_sb = wpool.tile([128, NB, 8, N], BF16)
    wqkv_sb = wpool.tile([128, 8, 3 * FS], BF16)
    wout_sb = wpool.tile([128, 2, 4, D], BF16)
    mask_sb = wpool.tile([128, 2, 2, 512], BF16)
    ident_sb = wpool.tile([128, 128], BF16)
    qkT_sb = wpool.tile([128, 2, NFLAT], BF16)          # [dims, q/k, b*N+i]
    # per j-tile [ones | junk | V_h]: row0=ones, rows 64:128 = V dims
    vaug_sb = wpool.tile([128, 32, HPC, 128], BF16)
    attr_sb = {h: wpool.tile([128, 4, ROWS], BF16, name=f"attr{h}")
               for h in range(HPC)}

    a2a_in = {h: dpool.tile([NC, HD, ROWS], BF16, name=f"a2ai{h}")
              for h in range(HPC)}
    a2a_out = {h: dpool.tile([NC, HD, ROWS], BF16, name=f"a2ao{h}")
               for h in range(HPC)}
    bar_in = dpool.tile([1, 16], F32, name="bar_in")
    bar_out = dpool.tile([NC, 16], F32, name="bar_out")

    # startup DMAs: first the two tiles the first real matmul needs, then
    # the rest staggered in consumption order so early tiles land early.
    nc.sync.dma_start(wqkv_sb[:, 0, :], wqkv[:, 0, :])
    nc.sync.dma_start(xt_sb[:, 0, 0, :], xT[:, 0, 0, :])
    nc.sync.dma_start(ident_sb[:], ident[:])
    nc.sync.dma_start(wqkv_sb[:, 1:8, :], wqkv[:, 1:8, :])
    for dt in range(1, 8):
        nc.sync.dma_start(xt_sb[:, 0, dt, :], xT[:, 0, dt, :])
    nc.sync.dma_start(mask_sb[:], mask[:])
    nc.vector.memset(vaug_sb[:, :, :, 0:1], 1.0)
    nc.vector.memset(vaug_sb[:, :, :, 1:64], 0.0)
    # explicit zero bias for Exp: avoids the shared const-0.0 SBUF tensor,
    # whose region aliases later pool tiles and trips false DMA/ACT races
    zbias = wpool.tile([128, 1], F32, name="zbias")
    nc.vector.memset(zbias[:], 0.0)
    bar_sb = wpool.tile([1, 16], F32, name="bar_sb")
    nc.vector.memset(bar_sb[:], 0.0)
    nc.sync.dma_start(bar_in[:], bar_sb[:])

    def qk_mm(ps, b, ft, ic, dt):
        nc.tensor.matmul(
            ps[:],
            wqkv_sb[:, dt, 128 * ft:128 * (ft + 1)],
            xt_sb[:, b, dt, 512 * ic:512 * (ic + 1)],
            start=(dt == 0), stop=(dt == 7))

    def vt_mm(ps, b, ic, dt):
        nc.tensor.matmul(
            ps[:],
            wqkv_sb[:, dt, 2 * FS:3 * FS],
            xt_sb[:, b, dt, 512 * ic:512 * (ic + 1)],
            start=(dt == 0), stop=(dt == 7))

    def finish_qk(ps, b, ft, ic):
        nc.vector.tensor_copy(
            qkT_sb[:, ft, b * N + 512 * ic: b * N + 512 * (ic + 1)], ps[:])

    def finish_v(vps_list, b, psum_pool, ptag, pbufs):
        vT_bf = spool.tile([128, N], BF16, tag="vtb", bufs=2, name=f"vtb{b}")
        for ic in range(4):
            nc.vector.tensor_copy(vT_bf[:, 512 * ic:512 * (ic + 1)],
                                  vps_list[ic][:])
        for it in range(16):
            tp = psum_pool.tile([128, 128], BF16, tag=ptag, bufs=pbufs,
                                name=f"t_ps{b}_{it}")
            nc.tensor.transpose(tp[:], vT_bf[:, 128 * it:128 * (it + 1)],
                                ident_sb[:])
            nc.vector.tensor_copy(
                vaug_sb[:, 16 * b + it, :, 64:128],
                tp[:].rearrange("p (h c) -> p h c", h=HPC))

    # ---- warmup while the xT DMA streams in ----
    # preload the ACT exp table (first use costs ~1.3us)
    wsc = spool.tile([128, 1], BF16, tag="wsc", name="wsc")
    nc.scalar.activation(wsc[:], zbias[:], AF.Exp, bias=zbias[:], scale=1.0)

    # ---- Phase 1, batch 0: dt-outer passes (overlap xT DMA) ----
    qk_ps = {(ft, ic): pinit.tile([128, 512], F32, tag="init",
                                  bufs=8, name=f"qk0_{ft}_{ic}")
             for ft in range(2) for ic in range(4)}
    for dt in range(8):
        for ft in range(2):
            for ic in range(4):
                qk_mm(qk_ps[ft, ic], 0, ft, ic, dt)
    for ft in range(2):
        for ic in range(4):
            finish_qk(qk_ps[ft, ic], 0, ft, ic)
    v_ps0 = [pinit.tile([128, 512], F32, tag="init", bufs=8,
                        name=f"v0_{ic}") for ic in range(4)]
    for dt in range(8):
        for ic in range(4):
            vt_mm(v_ps0[ic], 0, ic, dt)
    finish_v(v_ps0, 0, pinit, "init", 8)
    pinit_cm.__exit__(None, None, None)
    ppool_cm = tc.tile_pool(name="psum", bufs=1, space="PSUM")
    ppool = ppool_cm.__enter__()

    # batch-1 x and the output-projection weights: single big transfers,
    # needed only tens of us from now.
    nc.sync.dma_start(xt_sb[:, 1, :, :], xT[:, 1, :, :])
    nc.sync.dma_start(wout_sb[:], wout[:].rearrange("r p t i -> p r t i"))

    def p1_units(b):
        """phase1_seq(b) decomposed into single-matmul emission units so it
        can be interleaved into a pass2 as PE filler work. Ordered ic-major
        (31 units per ic) so a prefix makes i-chunk ic of batch b usable."""
        units = []
        state = {}

        def qk_group(ft, ic):
            def alloc():
                state[ft, ic] = ppool.tile([128, 512], F32, tag="mm", bufs=2,
                                           name=f"qk_ps{b}_{ft}_{ic}")
            for dt in range(8):
                def u(ft=ft, ic=ic, dt=dt):
                    if dt == 0:
                        alloc()
                    qk_mm(state[ft, ic], b, ft, ic, dt)
                units.append(u)
            units.append(lambda ft=ft, ic=ic: finish_qk(state[ft, ic], b, ft, ic))

        def v_group(ic):
            def alloc():
                state['v', ic] = ppool.tile([128, 512], F32, tag="mm", bufs=2,
                                            name=f"v_ps{b}_{ic}")
                if ic == 0:
                    state['vtb'] = spool.tile([128, N], BF16, tag="vtb",
                                              bufs=2, name=f"vtb{b}")
            for dt in range(8):
                def u(ic=ic, dt=dt):
                    if dt == 0:
                        alloc()
                    vt_mm(state['v', ic], b, ic, dt)
                units.append(u)

            def fin(ic=ic):
                nc.vector.tensor_copy(
                    state['vtb'][:, 512 * ic:512 * (ic + 1)],
                    state['v', ic][:])
            units.append(fin)

        def tr_unit(it):
            def tr(it=it):
                tp = ppool.tile([128, 128], BF16, tag="mm", bufs=2,
                                name=f"t_ps{b}_{it}")
                nc.tensor.transpose(tp[:], state['vtb'][:, 128 * it:128 * (it + 1)],
                                    ident_sb[:])
                nc.vector.tensor_copy(
                    vaug_sb[:, 16 * b + it, :, 64:128],
                    tp[:].rearrange("p (h c) -> p h c", h=HPC))
            units.append(tr)

        for ic in range(4):
            qk_group(0, ic)
            qk_group(1, ic)
            v_group(ic)
            for it in range(4 * ic, 4 * ic + 4):
                tr_unit(it)
        return units

    P1_UNITS_PER_IC = 31

    partialA = {}

    def roundA_units():
        """out-projection round A (head-A features): 8 psum groups, each a
        4-matmul accumulate + fp32 copy-out, as filler units."""
        units = []
        state = {}
        for it in range(4):
            for oc in range(2):
                def mm(it=it, oc=oc):
                    ps = ppool.tile([128, 512], F32, tag="mm", bufs=2,
                                    name=f"opA_{it}_{oc}")
                    state[it, oc] = ps
                    for t in range(4):
                        nc.tensor.matmul(
                            ps[:],
                            attr_sb[0][:, t, 128 * it:128 * (it + 1)],
                            wout_sb[:, 0, t, 512 * oc:512 * (oc + 1)],
                            start=(t == 0), stop=(t == 3))
                units.append(mm)

                def cp(it=it, oc=oc):
                    pa = spool.tile([128, 512], F32, tag="pA", bufs=8,
                                    name=f"pa{it}_{oc}")
                    nc.vector.tensor_copy(pa[:], state[it, oc][:])
                    partialA[it, oc] = pa
                units.append(cp)
        return units

    def normalize(h, b, ic, pv):
        # denom is pv row 0 (ones row of vaug)
        recip = spool.tile([1, 512], F32, tag="recip", bufs=2,
                           name=f"rc{h}_{b}_{ic}")
        nc.vector.reciprocal_approx_fast(recip[:], pv[0:1, :])
        bc = spool.tile([128, 512], F32, tag="bc", bufs=2,
                        name=f"bc{h}_{b}_{ic}")
        nc.gpsimd.partition_broadcast(bc[:], recip[:])
        anf = spool.tile([128, 512], BF16, tag="an", bufs=4,
                         name=f"an{h}_{b}_{ic}")
        nc.vector.tensor_mul(anf[64:128, :], pv[64:128, :], bc[64:128, :])
        nc.sync.dma_start(a2a_in[h][4 * b + ic], anf[64:128, :])

    def pass_batch(h, b, consume=None, at_chunk=None):
        """causal attention for head-pass h over all four 512-wide i-chunks
        of batch b, as one software-pipelined stream of j-tile PAIRS (each
        pair shares one exp/mask op; the PV of pair k is emitted after the
        S of pair k+1, across chunk boundaries). `consume()` emits PE
        filler work once per pair; `at_chunk(ic)` runs before each chunk's
        first pair (for prerequisite draining)."""
        plan = []
        for ic in range(4):
            for pr in range((4 * ic + 4) // 2):
                plan.append((ic, pr))
        pvs = {}

        def emit_pv(pend):
            pic, pj0, pcp, pptp = pend
            chunk_last = (pj0 // 2 == 2 * pic + 1)
            for jj in range(2):
                nc.tensor.matmul(pvs[pic][:, pcp:512],
                                 vaug_sb[:, 16 * b + pj0 + jj, h, :],
                                 pptp[:, jj, pcp:512],
                                 start=(pj0 + jj == 0),
                                 stop=(chunk_last and jj == 1))
            if chunk_last:
                normalize(h, b, pic, pvs[pic])

        pend = None
        for ic, pr in plan:
            if pr == 0:
                if at_chunk is not None:
                    at_chunk(ic)
                pvs[ic] = ppool.tile([128, 512], F32, tag="pv", bufs=2,
                                     name=f"pv{h}_{b}_{ic}")
            j0 = 2 * pr
            # pair base column: fully-masked columns below the j0 diagonal
            q0 = j0 - 4 * ic
            cp = 128 * q0 if q0 > 0 else 0
            sp = ppool.tile([128, 2, 512], F32, tag="s", bufs=2,
                            name=f"s{h}_{b}_{ic}_{pr}")
            ptp = ptpool.tile([128, 2, 512], BF16, tag="pt", bufs=6,
                              name=f"pt{h}_{b}_{ic}_{pr}")
            for jj in range(2):
                jt = j0 + jj
                nc.tensor.matmul(
                    sp[:, jj, cp:512],
                    qkT_sb[64 * h:64 * (h + 1), 1,
                           b * N + 128 * jt: b * N + 128 * (jt + 1)],
                    qkT_sb[64 * h:64 * (h + 1), 0,
                           b * N + 512 * ic + cp: b * N + 512 * (ic + 1)],
                    start=True, stop=True)
            nc.scalar.activation(ptp[:, :, cp:512], sp[:, :, cp:512],
                                 AF.Exp, bias=zbias[:], scale=SCALE)
            if q0 >= 0:
                # diagonal pair: mask both j-tiles in one multiply (the host
                # mask already zeroes each tile's fully-masked columns)
                nc.vector.tensor_mul(ptp[:, :, cp:512], ptp[:, :, cp:512],
                                     mask_sb[:, pr - 2 * ic, :, cp:512])
            if consume is not None:
                consume()
            if pend is not None:
                emit_pv(pend)
            pend = (ic, j0, cp, ptp)
        emit_pv(pend)

    def do_a2a(h):
        nc.gpsimd.collective_compute(
            "AllToAll", mybir.AluOpType.bypass,
            replica_groups=[list(range(NC))],
            ins=[a2a_in[h].opt()], outs=[a2a_out[h].opt()])
        # gather [8,64,512] -> [128 (u q), 4 t, 512]: tile t gets slots
        # 2t (partitions 0:64) and 2t+1 (partitions 64:128). SBUF side must
        # stay a plain partition-major AP; the slot shuffle lives on the
        # DRAM side ((u q) merges to one stride-512 dim). One DMA per
        # kt-tile so the first round matmuls (t=0) start before the whole
        # gather completes.
        for t in range(4):
            nc.sync.dma_start(
                attr_sb[h][:, t, :],
                a2a_out[h][2 * t:2 * t + 2].rearrange("u q i -> (u q) i"))

    # ---- Phase 2: pass A (head 2c), both batches ----
    # batch-1 QKV/V prep rides along as PE filler inside passA(b0); the
    # per-chunk prefix is force-drained just in time for passA(b1).
    units1 = p1_units(1)
    done1 = [0]

    def consume_n(k):
        while done1[0] < len(units1) and k > 0:
            units1[done1[0]]()
            done1[0] += 1
            k -= 1

    pass_batch(0, 0, consume=lambda: consume_n(3))

    def at_chunk_b1(ic):
        consume_n(max(0, P1_UNITS_PER_IC * (ic + 1) - done1[0]))

    pass_batch(0, 1, consume=lambda: consume_n(2), at_chunk=at_chunk_b1)
    consume_n(len(units1))
    do_a2a(0)

    # ---- pass B (head 2c+1) ----
    pass_batch(1, 0)
    # re-sync the cores while passB(b1) computes, so A2A#2's entry skew
    # (accumulated launch/throughput drift) is mostly absorbed here
    nc.gpsimd.collective_compute(
        "AllGather", mybir.AluOpType.bypass,
        replica_groups=[list(range(NC))],
        ins=[bar_in.opt()], outs=[bar_out.opt()])
    # out-projection round A rides along as PE filler inside passB(b1),
    # starting late enough that A2A#1 + its gather have surely landed
    unitsA = roundA_units()
    doneA = [0]
    pair_ctr = [0]

    def consume_late():
        pair_ctr[0] += 1
        if pair_ctr[0] > 8:
            for _ in range(2):
                if doneA[0] < len(unitsA):
                    unitsA[doneA[0]]()
                    doneA[0] += 1

    pass_batch(1, 1, consume=consume_late)
    while doneA[0] < len(unitsA):
        unitsA[doneA[0]]()
        doneA[0] += 1
    do_a2a(1)

    # ---- Phase 3: output projection round B + writeback ----
    for it in range(4):
        for oc in range(2):
            ps = ppool.tile([128, 512], F32, tag="mm", bufs=2,
                            name=f"opB_{it}_{oc}")
            for t in range(4):
                nc.tensor.matmul(
                    ps[:],
                    attr_sb[1][:, t, 128 * it:128 * (it + 1)],
                    wout_sb[:, 1, t, 512 * oc:512 * (oc + 1)],
                    start=(t == 0), stop=(t == 3))
            ob = spool.tile([128, 512], BF16, tag="ob", bufs=3,
                            name=f"ob{it}_{oc}")
            nc.vector.tensor_add(ob[:], ps[:], partialA[it, oc][:])
            nc.sync.dma_start(
                out[128 * it:128 * (it + 1), 512 * oc:512 * (oc + 1)], ob[:])

    for p in reversed(ctx_pools):
        p.__exit__(None, None, None)


def _host_inputs(x, w_qkv, w_out):
    x = np.asarray(x, dtype=np.float32)
    w_qkv = np.asarray(w_qkv, dtype=np.float32)
    w_out = np.asarray(w_out, dtype=np.float32)

    # xT[p, b, dt, i] = x[b, i, 128*dt + p]
    xTt = np.ascontiguousarray(
        x.transpose(2, 0, 1).reshape(8, 128, NB, N).transpose(1, 2, 0, 3)
    ).astype(ml_dtypes.bfloat16)

    wq, wk, wv = w_qkv[:, 0:D], w_qkv[:, D:2 * D], w_qkv[:, 2 * D:3 * D]

    # wout2[r, p, t, :]: round r, K-tile t holds w_out rows for heads
    # {4t + r} (p<64) and {4t + 2 + r} (p>=64)
    wout2 = np.empty((2, 128, 4, D), np.float32)
    for r in range(2):
        for t in range(4):
            wout2[r, 0:64, t, :] = w_out[256 * t + 64 * r:
                                         256 * t + 64 * r + 64, :]
            wout2[r, 64:128, t, :] = w_out[256 * t + 128 + 64 * r:
                                           256 * t + 128 + 64 * r + 64, :]
    wout2 = wout2.astype(ml_dtypes.bfloat16)

    # causal masks for the 4 diagonal j-tiles, paired: [p, pairq, jj, f]
    # keeps iff f >= p + 128*(2*pairq + jj)
    p = np.arange(128)[:, None]
    f = np.arange(512)[None, :]
    masks = np.stack(
        [np.stack([(f >= p + 128 * (2 * pq + jj)) for jj in range(2)], axis=1)
         for pq in range(2)], axis=1)
    masks = np.ascontiguousarray(masks).astype(ml_dtypes.bfloat16)
    identity = np.eye(128, dtype=ml_dtypes.bfloat16)

    in_maps = []
    for c in range(NC):
        sl = slice(FS * c, FS * (c + 1))
        wq_c = np.concatenate([wq[:, sl], wk[:, sl], wv[:, sl]], axis=1)
        wq_c = np.ascontiguousarray(
            wq_c.astype(ml_dtypes.bfloat16).reshape(8, 128, 3 * FS)
            .transpose(1, 0, 2))
        in_maps.append({
            "xT": xTt,
            "wqkv": wq_c,
            "wout": wout2,
            "mask": masks,
            "ident": identity,
        })
    return in_maps


def run_hw(inputs, trace=False, **kw):
    """Run on 8 NeuronCores. Returns (full_output, BassKernelResults)."""
    global _CACHED_NC
    if _CACHED_NC is None:
        _CACHED_NC = build_graph()
    in_maps = _host_inputs(inputs["x"], inputs["w_qkv"], inputs["w_out"])
    res = run_bass_kernel_spmd(_CACHED_NC, in_maps,
                               core_ids=list(range(NC)), trace=trace, **kw)
    y = np.concatenate([np.asarray(res.results[c]["out"]) for c in range(NC)],
                       axis=0).reshape(NB, N, D).astype(np.float32)
    return y, res


def kernel(**inputs):
    y, _ = run_hw(inputs, trace=bool(os.environ.get("BASS_TRACE")))
    return y

